# revision 10
# baseline (speedup 1.0000x reference)
"""Optimized fp16 variant of the hyperedge Maxmin decoder kernel.

Differences from kernel.py (fp32 baseline):
  - v_feat cast to fp16 on host: halves gather HBM traffic.
  - max/min trees run in fp16 (DVE 2x packed mode), two 128-hedge tiles
    fused per instruction to amortize instruction overhead.
  - the (max - min) subtract is folded into the PE: h = W1.T@mxT + (-W1).T@mnT.
  - indirect gathers batched 2 tiles per instruction (fewer SWDGE descriptor
    generation rounds on the Pool engine).
"""

import numpy as np

N_NODES = 100000
D = 128
H = 100000
S = 16
HID = 128
P = 128
N_CORES = 8
H_PER_CORE = H // N_CORES            # 12500
N_TILES = (H_PER_CORE + P - 1) // P  # 98
H_PAD = N_TILES * P                  # 12544
TPB = 2                              # tiles per block
N_BLOCKS = N_TILES // TPB            # 49


def build_nc(n_blocks=N_BLOCKS):
    from concourse import bass, bacc, mybir, tile
    from concourse.bass import IndirectOffsetOnAxis
    from concourse.masks import make_identity

    f32 = mybir.dt.float32
    f16 = mybir.dt.float16
    i32 = mybir.dt.int32
    Alu = mybir.AluOpType
    Act = mybir.ActivationFunctionType

    n_tiles = n_blocks * TPB
    hp = n_tiles * P
    W = TPB * S * D          # elems per partition per block (4096)

    nc = bacc.Bacc(
        "TRN2", target_bir_lowering=False, debug=False, num_devices=N_CORES
    )
    v_feat = nc.dram_tensor("v_feat", [N_NODES, D], f16, kind="ExternalInput").ap()
    hedge = nc.dram_tensor("hedge", [hp, S], i32, kind="ExternalInput").ap()
    w1 = nc.dram_tensor("w1", [D, HID], f16, kind="ExternalInput").ap()
    nw1 = nc.dram_tensor("nw1", [D, HID], f16, kind="ExternalInput").ap()
    b1 = nc.dram_tensor("b1", [HID], f32, kind="ExternalInput").ap()
    w2 = nc.dram_tensor("w2", [HID, 1], f16, kind="ExternalInput").ap()
    b2 = nc.dram_tensor("b2", [1], f32, kind="ExternalInput").ap()
    out = nc.dram_tensor("preds", [hp], f32, kind="ExternalOutput").ap()

    with tile.TileContext(nc) as tc:
        with (
            tc.tile_pool(name="const", bufs=1) as cpool,
            tc.tile_pool(name="emb", bufs=3) as epool,
            tc.tile_pool(name="work", bufs=2) as wpool,
            tc.tile_pool(name="small", bufs=3) as spool,
            tc.tile_pool(name="psum", bufs=2, space="PSUM") as ppool,
        ):
            w1_t = cpool.tile([D, HID], f16)
            nc.sync.dma_start(out=w1_t[:], in_=w1[:, :])
            nw1_t = cpool.tile([D, HID], f16)
            nc.sync.dma_start(out=nw1_t[:], in_=nw1[:, :])
            b1_t = cpool.tile([HID, 1], f32)
            nc.sync.dma_start(out=b1_t[:], in_=b1.rearrange("(p o) -> p o", o=1))
            w2_t = cpool.tile([HID, 1], f16)
            nc.sync.dma_start(out=w2_t[:], in_=w2[:, :])
            b2_t = cpool.tile([1, 1], f32)
            nc.sync.dma_start(out=b2_t[:], in_=b2.rearrange("(p o) -> p o", o=1))
            ident = cpool.tile([P, P], f16)
            make_identity(nc, ident[:])

            hedge_all = cpool.tile([P, n_tiles * S], i32)
            nc.sync.dma_start(
                out=hedge_all[:],
                in_=hedge.rearrange("(p n) s -> p (n s)", p=P),
            )

            preds = cpool.tile([1, hp], f32)

            for blk in range(n_blocks):
                emb = epool.tile([P, W], f16, tag="emb")
                # dummy Pool op: absorbs the slot-reuse WAR/WAW waits and the
                # hedge_all RAW so the indirect DMAs themselves need <=1 sync
                # wait (walrus direct2d limit)
                nc.gpsimd.tensor_copy(out=emb[0:1, 0:1], in_=hedge_all[0:1, 0:1])
                # walrus vector-indirect contract: ONE index per partition per
                # DMA; each gathers row hedge_all[p, slot] into partition p.
                for sl in range(TPB * S):
                    nc.gpsimd.indirect_dma_start(
                        out=emb[:, sl * D : (sl + 1) * D],
                        out_offset=None,
                        in_=v_feat[:, :],
                        in_offset=IndirectOffsetOnAxis(
                            ap=hedge_all[
                                :, blk * TPB * S + sl : blk * TPB * S + sl + 1
                            ],
                            axis=0,
                        ),
                    )
                # fused 2-tile binary trees (fp16, contiguous slices)
                # emb viewed per tile t at offset t*2048
                HB = S * D  # 2048 per tile

                def tree(op, tag):
                    l1 = wpool.tile([P, TPB * 8 * D], f16, tag=f"{tag}1")
                    nc.vector.tensor_tensor(
                        out=l1.rearrange("p (t x) -> p t x", t=TPB),
                        in0=emb.rearrange("p (t x) -> p t x", t=TPB)[:, :, : 8 * D],
                        in1=emb.rearrange("p (t x) -> p t x", t=TPB)[:, :, 8 * D :],
                        op=op,
                    )
                    l2 = wpool.tile([P, TPB * 4 * D], f16, tag=f"{tag}2")
                    nc.vector.tensor_tensor(
                        out=l2.rearrange("p (t x) -> p t x", t=TPB),
                        in0=l1.rearrange("p (t x) -> p t x", t=TPB)[:, :, : 4 * D],
                        in1=l1.rearrange("p (t x) -> p t x", t=TPB)[:, :, 4 * D :],
                        op=op,
                    )
                    l3 = wpool.tile([P, TPB * 2 * D], f16, tag=f"{tag}3")
                    nc.vector.tensor_tensor(
                        out=l3.rearrange("p (t x) -> p t x", t=TPB),
                        in0=l2.rearrange("p (t x) -> p t x", t=TPB)[:, :, : 2 * D],
                        in1=l2.rearrange("p (t x) -> p t x", t=TPB)[:, :, 2 * D :],
                        op=op,
                    )
                    l4 = wpool.tile([P, TPB * D], f16, tag=f"{tag}4")
                    nc.vector.tensor_tensor(
                        out=l4.rearrange("p (t x) -> p t x", t=TPB),
                        in0=l3.rearrange("p (t x) -> p t x", t=TPB)[:, :, :D],
                        in1=l3.rearrange("p (t x) -> p t x", t=TPB)[:, :, D:],
                        op=op,
                    )
                    return l4

                mx = tree(Alu.max, "a")
                mn = tree(Alu.min, "c")

                for t in range(TPB):
                    n = blk * TPB + t
                    mxT_ps = ppool.tile([P, P], f16, tag="mxT_ps")
                    nc.tensor.transpose(
                        out=mxT_ps[:], in_=mx[:, t * D : (t + 1) * D], identity=ident[:]
                    )
                    mnT_ps = ppool.tile([P, P], f16, tag="mnT_ps")
                    nc.tensor.transpose(
                        out=mnT_ps[:], in_=mn[:, t * D : (t + 1) * D], identity=ident[:]
                    )
                    mxT = spool.tile([P, P], f16, tag="mxT")
                    nc.scalar.copy(out=mxT[:], in_=mxT_ps[:])
                    mnT = spool.tile([P, P], f16, tag="mnT")
                    nc.scalar.copy(out=mnT[:], in_=mnT_ps[:])

                    h_ps = ppool.tile([HID, P], f32, tag="h_ps")
                    nc.tensor.matmul(
                        out=h_ps[:], lhsT=w1_t[:], rhs=mxT[:], start=True, stop=False
                    )
                    nc.tensor.matmul(
                        out=h_ps[:], lhsT=nw1_t[:], rhs=mnT[:], start=False, stop=True
                    )
                    hh = spool.tile([HID, P], f16, tag="hh")
                    nc.scalar.activation(
                        out=hh[:], in_=h_ps[:], func=Act.Relu, bias=b1_t[:, :1]
                    )

                    p_ps = ppool.tile([1, P], f32, tag="p_ps")
                    nc.tensor.matmul(
                        out=p_ps[:], lhsT=w2_t[:], rhs=hh[:], start=True, stop=True
                    )
                    nc.scalar.activation(
                        out=preds[0:1, n * P : (n + 1) * P],
                        in_=p_ps[:],
                        func=Act.Sigmoid,
                        bias=b2_t[:, :1],
                    )

            nc.sync.dma_start(
                out=out.rearrange("(o f) -> o f", o=1), in_=preds[0:1, :]
            )

    nc.compile()
    return nc


_NC_CACHE = {}


def _get_nc(n_blocks=N_BLOCKS):
    if n_blocks not in _NC_CACHE:
        _NC_CACHE[n_blocks] = build_nc(n_blocks)
    return _NC_CACHE[n_blocks]


def _shard_inputs(v_feat, hedge_info, W1, b1, W2, b2):
    v = np.ascontiguousarray(np.asarray(v_feat, dtype=np.float32).astype(np.float16))
    he = np.asarray(hedge_info).astype(np.int32)
    w1 = np.ascontiguousarray(np.asarray(W1, dtype=np.float32).astype(np.float16))
    nw1 = np.ascontiguousarray(-w1)
    b1_ = np.ascontiguousarray(np.asarray(b1, dtype=np.float32))
    w2 = np.ascontiguousarray(np.asarray(W2, dtype=np.float32).astype(np.float16))
    b2_ = np.ascontiguousarray(np.asarray(b2, dtype=np.float32))

    in_maps = []
    for c in range(N_CORES):
        shard = he[c * H_PER_CORE : (c + 1) * H_PER_CORE]
        if shard.shape[0] < H_PAD:
            pad = np.zeros((H_PAD - shard.shape[0], S), np.int32)
            shard = np.concatenate([shard, pad], axis=0)
        shard = np.ascontiguousarray(
            shard.reshape(N_TILES, P, S).transpose(1, 0, 2).reshape(H_PAD, S)
        )
        in_maps.append(
            {
                "v_feat": v,
                "hedge": shard,
                "w1": w1,
                "nw1": nw1,
                "b1": b1_,
                "w2": w2,
                "b2": b2_,
            }
        )
    return in_maps


def run_sharded(v_feat, hedge_info, W1, b1, W2, b2, trace=False):
    from concourse.bass_utils import run_bass_kernel_spmd

    nc = _get_nc()
    in_maps = _shard_inputs(v_feat, hedge_info, W1, b1, W2, b2)
    res = run_bass_kernel_spmd(
        nc, in_maps, core_ids=list(range(N_CORES)), trace=trace
    )
    parts = [res.results[c]["preds"][:H_PER_CORE] for c in range(N_CORES)]
    preds = np.concatenate(parts, axis=0).reshape(H, 1).astype(np.float32)
    return preds, res


def kernel(v_feat, hedge_info, W1, b1, W2, b2):
    preds, _ = run_sharded(v_feat, hedge_info, W1, b1, W2, b2, trace=False)
    return preds


# revision 11
# speedup vs baseline: 1.0064x; 1.0064x over previous
"""Trainium2 Bass kernel for the hyperedge Maxmin-pool decoder.

reference:
    emb = v_feat[hedge_info]                  # [H, S, D] gather
    agg = emb.max(1) - emb.min(1)             # [H, D]
    h = relu(agg @ W1 + b1)                   # [H, HID]
    preds = sigmoid(h @ W2 + b2)              # [H, 1]

Data-parallel over hyperedges across 8 NeuronCores; v_feat and MLP weights
replicated. Per core, per tile of 128 hyperedges:
  - 16 indirect-DMA row gathers (the toolchain's vector-indirect contract is
    ONE index per partition per DMA), fp16 table (host-cast) to halve traffic,
  - DVE binary max/min trees in fp16 packed mode, 2 tiles fused per op,
  - (max - min) folded into the PE via W1.T@mxT + (-W1).T@mnT accumulation,
  - relu/sigmoid with fused per-partition bias on ACT; one final output DMA.
"""

import numpy as np

N_NODES = 100000
D = 128
H = 100000
S = 16
HID = 128
P = 128
N_CORES = 8
H_PER_CORE = H // N_CORES            # 12500
N_TILES = (H_PER_CORE + P - 1) // P  # 98
H_PAD = N_TILES * P                  # 12544
TPB = 2                              # tiles per block
N_BLOCKS = N_TILES // TPB            # 49


def build_nc(n_blocks=N_BLOCKS):
    from concourse import bass, bacc, mybir, tile
    from concourse.bass import IndirectOffsetOnAxis
    from concourse.masks import make_identity

    f32 = mybir.dt.float32
    f16 = mybir.dt.float16
    i32 = mybir.dt.int32
    Alu = mybir.AluOpType
    Act = mybir.ActivationFunctionType

    n_tiles = n_blocks * TPB
    hp = n_tiles * P
    W = TPB * S * D          # elems per partition per block (4096)

    nc = bacc.Bacc(
        "TRN2", target_bir_lowering=False, debug=False, num_devices=N_CORES
    )
    v_feat = nc.dram_tensor("v_feat", [N_NODES, D], f16, kind="ExternalInput").ap()
    hedge = nc.dram_tensor("hedge", [hp, S], i32, kind="ExternalInput").ap()
    w1 = nc.dram_tensor("w1", [D, HID], f16, kind="ExternalInput").ap()
    nw1 = nc.dram_tensor("nw1", [D, HID], f16, kind="ExternalInput").ap()
    b1 = nc.dram_tensor("b1", [HID], f32, kind="ExternalInput").ap()
    w2 = nc.dram_tensor("w2", [HID, 1], f16, kind="ExternalInput").ap()
    b2 = nc.dram_tensor("b2", [1], f32, kind="ExternalInput").ap()
    out = nc.dram_tensor("preds", [hp], f32, kind="ExternalOutput").ap()

    with tile.TileContext(nc) as tc:
        with (
            tc.tile_pool(name="const", bufs=1) as cpool,
            tc.tile_pool(name="emb", bufs=4) as epool,
            tc.tile_pool(name="work", bufs=2) as wpool,
            tc.tile_pool(name="small", bufs=4) as spool,
            tc.tile_pool(name="psum", bufs=2, space="PSUM") as ppool,
        ):
            w1_t = cpool.tile([D, HID], f16)
            nc.sync.dma_start(out=w1_t[:], in_=w1[:, :])
            nw1_t = cpool.tile([D, HID], f16)
            nc.sync.dma_start(out=nw1_t[:], in_=nw1[:, :])
            b1_t = cpool.tile([HID, 1], f32)
            nc.sync.dma_start(out=b1_t[:], in_=b1.rearrange("(p o) -> p o", o=1))
            w2_t = cpool.tile([HID, 1], f16)
            nc.sync.dma_start(out=w2_t[:], in_=w2[:, :])
            b2_t = cpool.tile([1, 1], f32)
            nc.sync.dma_start(out=b2_t[:], in_=b2.rearrange("(p o) -> p o", o=1))
            ident = cpool.tile([P, P], f16)
            make_identity(nc, ident[:])

            hedge_all = cpool.tile([P, n_tiles * S], i32)
            nc.sync.dma_start(
                out=hedge_all[:],
                in_=hedge.rearrange("(p n) s -> p (n s)", p=P),
            )

            preds = cpool.tile([1, hp], f32)

            for blk in range(n_blocks):
                emb = epool.tile([P, W], f16, tag="emb")
                # dummy Pool op: absorbs the slot-reuse WAR/WAW waits and the
                # hedge_all RAW so the indirect DMAs themselves need <=1 sync
                # wait (walrus direct2d limit)
                nc.gpsimd.tensor_copy(out=emb[0:1, 0:1], in_=hedge_all[0:1, 0:1])
                # walrus vector-indirect contract: ONE index per partition per
                # DMA; each gathers row hedge_all[p, slot] into partition p.
                for sl in range(TPB * S):
                    nc.gpsimd.indirect_dma_start(
                        out=emb[:, sl * D : (sl + 1) * D],
                        out_offset=None,
                        in_=v_feat[:, :],
                        in_offset=IndirectOffsetOnAxis(
                            ap=hedge_all[
                                :, blk * TPB * S + sl : blk * TPB * S + sl + 1
                            ],
                            axis=0,
                        ),
                    )
                # fused 2-tile binary trees (fp16, contiguous slices)
                # emb viewed per tile t at offset t*2048
                HB = S * D  # 2048 per tile

                def tree(op, tag):
                    l1 = wpool.tile([P, TPB * 8 * D], f16, tag=f"{tag}1")
                    nc.vector.tensor_tensor(
                        out=l1.rearrange("p (t x) -> p t x", t=TPB),
                        in0=emb.rearrange("p (t x) -> p t x", t=TPB)[:, :, : 8 * D],
                        in1=emb.rearrange("p (t x) -> p t x", t=TPB)[:, :, 8 * D :],
                        op=op,
                    )
                    l2 = wpool.tile([P, TPB * 4 * D], f16, tag=f"{tag}2")
                    nc.vector.tensor_tensor(
                        out=l2.rearrange("p (t x) -> p t x", t=TPB),
                        in0=l1.rearrange("p (t x) -> p t x", t=TPB)[:, :, : 4 * D],
                        in1=l1.rearrange("p (t x) -> p t x", t=TPB)[:, :, 4 * D :],
                        op=op,
                    )
                    l3 = wpool.tile([P, TPB * 2 * D], f16, tag=f"{tag}3")
                    nc.vector.tensor_tensor(
                        out=l3.rearrange("p (t x) -> p t x", t=TPB),
                        in0=l2.rearrange("p (t x) -> p t x", t=TPB)[:, :, : 2 * D],
                        in1=l2.rearrange("p (t x) -> p t x", t=TPB)[:, :, 2 * D :],
                        op=op,
                    )
                    l4 = wpool.tile([P, TPB * D], f16, tag=f"{tag}4")
                    nc.vector.tensor_tensor(
                        out=l4.rearrange("p (t x) -> p t x", t=TPB),
                        in0=l3.rearrange("p (t x) -> p t x", t=TPB)[:, :, :D],
                        in1=l3.rearrange("p (t x) -> p t x", t=TPB)[:, :, D:],
                        op=op,
                    )
                    return l4

                mx = tree(Alu.max, "a")
                mn = tree(Alu.min, "c")

                for t in range(TPB):
                    n = blk * TPB + t
                    mxT_ps = ppool.tile([P, P], f16, tag="mxT_ps")
                    nc.tensor.transpose(
                        out=mxT_ps[:], in_=mx[:, t * D : (t + 1) * D], identity=ident[:]
                    )
                    mnT_ps = ppool.tile([P, P], f16, tag="mnT_ps")
                    nc.tensor.transpose(
                        out=mnT_ps[:], in_=mn[:, t * D : (t + 1) * D], identity=ident[:]
                    )
                    mxT = spool.tile([P, P], f16, tag="mxT")
                    nc.scalar.copy(out=mxT[:], in_=mxT_ps[:])
                    mnT = spool.tile([P, P], f16, tag="mnT")
                    nc.scalar.copy(out=mnT[:], in_=mnT_ps[:])

                    h_ps = ppool.tile([HID, P], f32, tag="h_ps")
                    nc.tensor.matmul(
                        out=h_ps[:], lhsT=w1_t[:], rhs=mxT[:], start=True, stop=False
                    )
                    nc.tensor.matmul(
                        out=h_ps[:], lhsT=nw1_t[:], rhs=mnT[:], start=False, stop=True
                    )
                    hh = spool.tile([HID, P], f16, tag="hh")
                    nc.scalar.activation(
                        out=hh[:], in_=h_ps[:], func=Act.Relu, bias=b1_t[:, :1]
                    )

                    p_ps = ppool.tile([1, P], f32, tag="p_ps")
                    nc.tensor.matmul(
                        out=p_ps[:], lhsT=w2_t[:], rhs=hh[:], start=True, stop=True
                    )
                    nc.scalar.activation(
                        out=preds[0:1, n * P : (n + 1) * P],
                        in_=p_ps[:],
                        func=Act.Sigmoid,
                        bias=b2_t[:, :1],
                    )

            nc.sync.dma_start(
                out=out.rearrange("(o f) -> o f", o=1), in_=preds[0:1, :]
            )

    nc.compile()
    return nc


_NC_CACHE = {}


def _get_nc(n_blocks=N_BLOCKS):
    if n_blocks not in _NC_CACHE:
        _NC_CACHE[n_blocks] = build_nc(n_blocks)
    return _NC_CACHE[n_blocks]


def _shard_inputs(v_feat, hedge_info, W1, b1, W2, b2):
    v = np.ascontiguousarray(np.asarray(v_feat, dtype=np.float32).astype(np.float16))
    he = np.asarray(hedge_info).astype(np.int32)
    w1 = np.ascontiguousarray(np.asarray(W1, dtype=np.float32).astype(np.float16))
    nw1 = np.ascontiguousarray(-w1)
    b1_ = np.ascontiguousarray(np.asarray(b1, dtype=np.float32))
    w2 = np.ascontiguousarray(np.asarray(W2, dtype=np.float32).astype(np.float16))
    b2_ = np.ascontiguousarray(np.asarray(b2, dtype=np.float32))

    in_maps = []
    for c in range(N_CORES):
        shard = he[c * H_PER_CORE : (c + 1) * H_PER_CORE]
        if shard.shape[0] < H_PAD:
            pad = np.zeros((H_PAD - shard.shape[0], S), np.int32)
            shard = np.concatenate([shard, pad], axis=0)
        shard = np.ascontiguousarray(
            shard.reshape(N_TILES, P, S).transpose(1, 0, 2).reshape(H_PAD, S)
        )
        in_maps.append(
            {
                "v_feat": v,
                "hedge": shard,
                "w1": w1,
                "nw1": nw1,
                "b1": b1_,
                "w2": w2,
                "b2": b2_,
            }
        )
    return in_maps


def run_sharded(v_feat, hedge_info, W1, b1, W2, b2, trace=False):
    from concourse.bass_utils import run_bass_kernel_spmd

    nc = _get_nc()
    in_maps = _shard_inputs(v_feat, hedge_info, W1, b1, W2, b2)
    res = run_bass_kernel_spmd(
        nc, in_maps, core_ids=list(range(N_CORES)), trace=trace
    )
    parts = [res.results[c]["preds"][:H_PER_CORE] for c in range(N_CORES)]
    preds = np.concatenate(parts, axis=0).reshape(H, 1).astype(np.float32)
    return preds, res


def kernel(v_feat, hedge_info, W1, b1, W2, b2):
    preds, _ = run_sharded(v_feat, hedge_info, W1, b1, W2, b2, trace=False)
    return preds
